# revision 9
# baseline (speedup 1.0000x reference)
"""Trainium2 Bass kernel for nn_Adaptive_Att (GNN edge attention logits).

Math: out[e] = sigmoid(x[row[e]] @ w_row + x[col[e]] @ w_col), [E, 1] f32.

Strategy (8 NeuronCores, edge-parallel):
  - Host shards nodes 8-ways; each core computes s = [x@w_row | x@w_col] for
    its 12544-node shard on the TensorEngine, then an AllGather replicates the
    full per-node score tables (2 x 100352 f32) to every core.
  - The tables are laid into SBUF as 8 sub-tables of 25088 entries (4 row
    chunks + 4 col chunks), one per partition mod 8, so a single GPSIMD
    ap_gather instruction gathers all 4 candidate chunks for every edge of
    every 16-partition group at once.
  - Host pre-encodes each edge endpoint as (chunk, local_index<25088) plus
    uint8 one-hot chunk masks; the DVE resolves candidates with
    copy_predicated chains, adds row+col parts, and the ACT engine applies
    sigmoid. Edges are sharded 200K per core, 25088 per gather group.
"""

import os
import numpy as np

H = 128
N_NODES = 100000
N_EDGES = 1600000
N_CORES = 8
SHARD = 12544              # nodes per core for the s-precompute
NTOT = SHARD * N_CORES     # 100352 (padded node count)
CH = 25088                 # sub-table chunk size = 2 shards
EPC = N_EDGES // N_CORES   # 200000 edges per core
GEDGE = 25088              # padded edges per gather group (8 groups/core)
PADE = GEDGE * 8           # 200704 padded edges per core
TIL = 16                   # gather tiles per group
TG = GEDGE // TIL          # 1568 edges per tile
TGC = TG // 16             # 98 idx columns per tile
MMT = 512                  # matmul tile (psum bank)

LAST_EXEC_NS = None
_CACHE = {}


def _build():
    import concourse.bass as bass
    import concourse.mybir as mybir
    from concourse.bacc import Bacc
    from contextlib import ExitStack

    f32 = mybir.dt.float32
    nc = Bacc()

    xT_ext = nc.declare_dram_parameter("xT", [H, SHARD], f32, isOutput=False)
    w_ext = nc.declare_dram_parameter("w", [H, 2], f32, isOutput=False)
    idxr_ext = nc.declare_dram_parameter("idxr", [128, TGC * TIL], mybir.dt.int16, isOutput=False)
    idxc_ext = nc.declare_dram_parameter("idxc", [128, TGC * TIL], mybir.dt.int16, isOutput=False)
    mask_ext = nc.declare_dram_parameter("mask", [6, 8, GEDGE], mybir.dt.uint8, isOutput=False)
    out_ext = nc.declare_dram_parameter("out", [8, GEDGE], f32, isOutput=True)

    s_loc = nc.dram_tensor("s_loc", [2, SHARD], f32)
    s_all = nc.dram_tensor("s_all", [16, SHARD], f32, addr_space="Shared")

    es = ExitStack()
    # big: phase A -> xT shard ([:, :SHARD]) + s staging (rows 0-1, cols SHARD:2*SHARD)
    #      phase B -> the gather table ([:, :CH], partition p holds sub-table p%8)
    big = es.enter_context(nc.sbuf_tensor([128, 25600], f32))
    w_sb = es.enter_context(nc.sbuf_tensor([128, 2], f32))
    idxr_sb = es.enter_context(nc.sbuf_tensor([128, TGC * TIL], mybir.dt.int16))
    idxc_sb = es.enter_context(nc.sbuf_tensor([128, TGC * TIL], mybir.dt.int16))
    outR = [es.enter_context(nc.sbuf_tensor(f"outR{i}", [128, TG], f32)) for i in range(4)]
    outC = [es.enter_context(nc.sbuf_tensor(f"outC{i}", [128, TG], f32)) for i in range(4)]
    blkR = es.enter_context(nc.sbuf_tensor([128, TG], f32))
    blkC = es.enter_context(nc.sbuf_tensor([128, TG], f32))
    mR = es.enter_context(nc.sbuf_tensor([128, TG], mybir.dt.uint8))
    mC = es.enter_context(nc.sbuf_tensor([128, TG], mybir.dt.uint8))
    sig = [es.enter_context(nc.sbuf_tensor(f"sig{i}", [8, TG], f32)) for i in range(2)]
    psum = [es.enter_context(nc.psum_tensor(f"psum{i}", [2, MMT], f32)) for i in range(2)]

    xT_view = big[:, :SHARD]
    s_view = big[0:2, SHARD:2 * SHARD]
    tab_view = big[:, :CH]

    n_mm = (SHARD + MMT - 1) // MMT  # 25 (24x512 + 256)

    with (
        nc.semaphore("is_") as is_,     # input DMAs
        nc.semaphore("pe") as pe,       # matmul tiles
        nc.semaphore("ve") as ve,       # psum evacuations
        nc.semaphore("sl") as sl,       # s -> dram
        nc.semaphore("cc") as cc,       # collective
        nc.semaphore("ts") as ts,       # table DMAs
        nc.semaphore("gs") as gs,       # gathers
        nc.semaphore("rs") as rs,       # reshuffle+mask DMAs
        nc.semaphore("vs") as vs,       # DVE combine per tile
        nc.semaphore("asm") as asm,     # ACT sigmoid per tile
        nc.semaphore("os_") as os_,     # output DMAs
        nc.Block() as block,
    ):
        @block.sync
        def _(sync):
            sync.dma_start(out=xT_view, in_=xT_ext[:]).then_inc(is_, 16)
            sync.dma_start(out=w_sb[:], in_=w_ext[:]).then_inc(is_, 16)
            sync.dma_start(out=idxr_sb[:], in_=idxr_ext[:]).then_inc(is_, 16)
            sync.dma_start(out=idxc_sb[:], in_=idxc_ext[:]).then_inc(is_, 16)

            # s table to DRAM once computed
            sync.wait_ge(ve, n_mm)
            sync.dma_start(out=s_loc[:], in_=s_view).then_inc(sl, 16)

            # after the collective, broadcast sub-tables into SBUF:
            # partition p holds sub-table c=p%8; c<4: row chunk c (s_all rows
            # 4c, 4c+2); c>=4: col chunk c-4 (s_all rows 4(c-4)+1, 4(c-4)+3).
            sync.wait_ge(cc, 1)
            from concourse.ap import AP as _AP
            for c in range(8):
                r0 = 4 * c if c < 4 else 4 * (c - 4) + 1
                src = _AP(
                    s_all[:].tensor, r0 * SHARD,
                    [[0, 16], [2 * SHARD, 2], [1, SHARD]],
                )
                dst = big[c::8, :CH].rearrange("p (a b) -> p a b", a=2)
                sync.dma_start(out=dst, in_=src).then_inc(ts, 16)

            # steady-state tile loop
            for t in range(TIL):
                b = t % 4
                # output DMA for tile t-1
                if t > 0:
                    sync.wait_ge(asm, t)
                    sync.dma_start(
                        out=out_ext[:, TG * (t - 1):TG * t], in_=sig[(t - 1) % 2][:]
                    ).then_inc(os_, 16)
                # reshuffle candidates of tile t into 32-aligned blocks
                sync.wait_ge(gs, 2 * (t + 1))
                if t > 0:
                    sync.wait_ge(asm, t)  # blk/mask buffers free after ACT(t-1)
                for c in range(4):
                    sync.dma_start(out=blkR[32 * c:32 * c + 8, :], in_=outR[b][c::16, :]).then_inc(rs, 16)
                    sync.dma_start(out=blkC[32 * c:32 * c + 8, :], in_=outC[b][(4 + c)::16, :]).then_inc(rs, 16)
                for c in range(1, 4):
                    sync.dma_start(out=mR[32 * c:32 * c + 8, :], in_=mask_ext[c - 1, :, TG * t:TG * (t + 1)]).then_inc(rs, 16)
                    sync.dma_start(out=mC[32 * c:32 * c + 8, :], in_=mask_ext[c + 2, :, TG * t:TG * (t + 1)]).then_inc(rs, 16)
            # final output DMA
            sync.wait_ge(asm, TIL)
            sync.dma_start(
                out=out_ext[:, TG * (TIL - 1):TG * TIL], in_=sig[(TIL - 1) % 2][:]
            ).then_inc(os_, 16)
            sync.wait_ge(os_, 16 * TIL)

        @block.tensor
        def _(tensor):
            tensor.wait_ge(is_, 64)
            for k in range(n_mm):
                lo = MMT * k
                hi = min(SHARD, lo + MMT)
                if k >= 2:
                    tensor.wait_ge(ve, k - 1)
                tensor.matmul(
                    psum[k % 2][:, :hi - lo], w_sb[:], xT_view[:, lo:hi],
                    start=True, stop=True,
                ).then_inc(pe, 1)

        @block.vector
        def _(vector):
            for k in range(n_mm):
                lo = MMT * k
                hi = min(SHARD, lo + MMT)
                vector.wait_ge(pe, k + 1)
                vector.tensor_copy(s_view[:, lo:hi], psum[k % 2][:, :hi - lo]).then_inc(ve, 1)
            # combine loop
            for t in range(TIL):
                vector.wait_ge(rs, 224 * (t + 1))
                for c in range(1, 4):
                    vector.copy_predicated(blkR[0:8, :], mR[32 * c:32 * c + 8, :], blkR[32 * c:32 * c + 8, :])
                    vector.copy_predicated(blkC[0:8, :], mC[32 * c:32 * c + 8, :], blkC[32 * c:32 * c + 8, :])
                vector.tensor_tensor(
                    out=blkR[0:8, :], in0=blkR[0:8, :], in1=blkC[0:8, :],
                    op=mybir.AluOpType.add,
                ).then_inc(vs, 1)

        @block.scalar
        def _(scalar):
            for t in range(TIL):
                scalar.wait_ge(vs, t + 1)
                if t >= 2:
                    scalar.wait_ge(os_, 16 * (t - 1))
                scalar.activation(
                    sig[t % 2][:], blkR[0:8, :], mybir.ActivationFunctionType.Sigmoid
                ).then_inc(asm, 1)

        @block.gpsimd
        def _(gpsimd):
            gpsimd.wait_ge(sl, 16)
            gpsimd.collective_compute(
                "AllGather",
                mybir.AluOpType.bypass,
                replica_groups=[list(range(N_CORES))],
                ins=[s_loc[:]],
                outs=[s_all[:]],
            ).then_inc(cc, 1)
            gpsimd.wait_ge(ts, 16 * 8)
            gpsimd.wait_ge(is_, 64)
            for t in range(TIL):
                b = t % 4
                if t >= 4:
                    gpsimd.wait_ge(rs, 224 * (t - 3))
                gpsimd.ap_gather(
                    outR[b][:], tab_view, idxr_sb[:, TGC * t:TGC * (t + 1)],
                    channels=128, num_elems=CH, d=1, num_idxs=TG,
                ).then_inc(gs, 1)
                gpsimd.ap_gather(
                    outC[b][:], tab_view, idxc_sb[:, TGC * t:TGC * (t + 1)],
                    channels=128, num_elems=CH, d=1, num_idxs=TG,
                ).then_inc(gs, 1)

    nc.finalize()
    return nc


def _wrap(arr):
    # [8, GEDGE] per-group streams -> ap_gather wrapped layout [128, GEDGE//16]
    return (
        arr.reshape(8, GEDGE // 16, 16).transpose(0, 2, 1).reshape(128, GEDGE // 16)
    )


def kernel(edge_index, x, att_weight):
    global LAST_EXEC_NS
    from concourse.bass_utils import run_bass_kernel_spmd

    if "nc" not in _CACHE:
        _CACHE["nc"] = _build()
    nc = _CACHE["nc"]

    x = np.asarray(x, dtype=np.float32)
    att_weight = np.asarray(att_weight, dtype=np.float32)
    ei = np.asarray(edge_index)

    xT = np.zeros((H, NTOT), dtype=np.float32)
    xT[:, :N_NODES] = x.T
    w = np.empty((H, 2), dtype=np.float32)
    w[:, 0] = att_weight[0, :H]
    w[:, 1] = att_weight[0, H:]

    in_maps = []
    for c in range(N_CORES):
        sl = slice(EPC * c, EPC * (c + 1))
        r = np.zeros(PADE, dtype=np.int64)
        co = np.zeros(PADE, dtype=np.int64)
        r[:EPC] = ei[0, sl]
        co[:EPC] = ei[1, sl]
        rch = (r // CH).astype(np.uint8).reshape(8, GEDGE)
        cch = (co // CH).astype(np.uint8).reshape(8, GEDGE)
        rloc = (r % CH).astype(np.int16).reshape(8, GEDGE)
        cloc = (co % CH).astype(np.int16).reshape(8, GEDGE)
        mask = np.zeros((6, 8, GEDGE), dtype=np.uint8)
        for cc_ in range(1, 4):
            mask[cc_ - 1] = rch == cc_
            mask[cc_ + 2] = cch == cc_
        in_maps.append({
            "xT": np.ascontiguousarray(xT[:, SHARD * c:SHARD * (c + 1)]),
            "w": w,
            "idxr": np.ascontiguousarray(_wrap(rloc)),
            "idxc": np.ascontiguousarray(_wrap(cloc)),
            "mask": mask,
        })

    trace = bool(os.environ.get("BASS_TRACE"))
    try:
        res = run_bass_kernel_spmd(nc, in_maps, list(range(N_CORES)), trace=trace)
    except Exception:
        if not trace:
            raise
        res = run_bass_kernel_spmd(nc, in_maps, list(range(N_CORES)), trace=False)
    LAST_EXEC_NS = res.exec_time_ns

    parts = [res.results[c]["out"].reshape(-1)[:EPC] for c in range(N_CORES)]
    return np.concatenate(parts).astype(np.float32)[:, None]


# revision 12
# speedup vs baseline: 1.1867x; 1.1867x over previous
"""Trainium2 Bass kernel for nn_Adaptive_Att (GNN edge attention logits).

Math: out[e] = sigmoid(x[row[e]] @ w_row + x[col[e]] @ w_col), [E, 1] f32.

Strategy (8 NeuronCores, edge-parallel):
  - Host shards nodes 8-ways; each core computes s = [x@w_row | x@w_col] for
    its 12544-node shard on the TensorEngine, then an AllGather replicates the
    full per-node score tables (2 x 100352 f32) to every core.
  - The tables are laid into SBUF as 8 sub-tables of 25088 entries (4 row
    chunks + 4 col chunks), one per partition mod 8, so a single GPSIMD
    ap_gather instruction gathers all 4 candidate chunks for every edge of
    every 16-partition group at once.
  - Host pre-encodes each edge endpoint as (chunk, local_index<25088) plus
    uint8 one-hot chunk masks; the DVE resolves candidates with
    copy_predicated chains, adds row+col parts, and the ACT engine applies
    sigmoid. Edges are sharded 200K per core, 25088 per gather group.
"""

import os
import numpy as np

H = 128
N_NODES = 100000
N_EDGES = 1600000
N_CORES = 8
SHARD = 12544              # nodes per core for the s-precompute
NTOT = SHARD * N_CORES     # 100352 (padded node count)
CH = 25088                 # sub-table chunk size = 2 shards
EPC = N_EDGES // N_CORES   # 200000 edges per core
GEDGE = 25088              # padded edges per gather group (8 groups/core)
PADE = GEDGE * 8           # 200704 padded edges per core
TIL = 16                   # gather tiles per group
TG = GEDGE // TIL          # 1568 edges per tile
TGC = TG // 16             # 98 idx columns per tile
MMT = 512                  # matmul tile (psum bank)

LAST_EXEC_NS = None
_CACHE = {}


def _build():
    import concourse.bass as bass
    import concourse.mybir as mybir
    from concourse.bacc import Bacc
    from contextlib import ExitStack

    f32 = mybir.dt.float32
    nc = Bacc()

    xT_ext = nc.declare_dram_parameter("xT", [H, SHARD], f32, isOutput=False)
    w_ext = nc.declare_dram_parameter("w", [H, 2], f32, isOutput=False)
    idxr_ext = nc.declare_dram_parameter("idxr", [128, TGC * TIL], mybir.dt.int16, isOutput=False)
    idxc_ext = nc.declare_dram_parameter("idxc", [128, TGC * TIL], mybir.dt.int16, isOutput=False)
    mask_ext = nc.declare_dram_parameter("mask", [6, 8, GEDGE], mybir.dt.uint8, isOutput=False)
    out_ext = nc.declare_dram_parameter("out", [8, GEDGE], f32, isOutput=True)

    s_loc = nc.dram_tensor("s_loc", [2, SHARD], f32)
    s_all = nc.dram_tensor("s_all", [16, SHARD], f32, addr_space="Shared")

    es = ExitStack()
    # big: phase A -> xT shard ([:, :SHARD]) + s staging (rows 0-1, cols SHARD:2*SHARD)
    #      phase B -> the gather table ([:, :CH], partition p holds sub-table p%8)
    big = es.enter_context(nc.sbuf_tensor([128, 25600], f32))
    w_sb = es.enter_context(nc.sbuf_tensor([128, 2], f32))
    idxr_sb = es.enter_context(nc.sbuf_tensor([128, TGC * TIL], mybir.dt.int16))
    idxc_sb = es.enter_context(nc.sbuf_tensor([128, TGC * TIL], mybir.dt.int16))
    outR = [es.enter_context(nc.sbuf_tensor(f"outR{i}", [128, TG], f32)) for i in range(2)]
    outC = [es.enter_context(nc.sbuf_tensor(f"outC{i}", [128, TG], f32)) for i in range(2)]
    blkR = es.enter_context(nc.sbuf_tensor([128, TG], f32))
    blkC = es.enter_context(nc.sbuf_tensor([128, TG], f32))
    mR = [es.enter_context(nc.sbuf_tensor(f"mR{i}", [128, TG], mybir.dt.uint8)) for i in range(2)]
    mC = [es.enter_context(nc.sbuf_tensor(f"mC{i}", [128, TG], mybir.dt.uint8)) for i in range(2)]
    sig = [es.enter_context(nc.sbuf_tensor(f"sig{i}", [8, TG], f32)) for i in range(2)]
    psum = [es.enter_context(nc.psum_tensor(f"psum{i}", [2, MMT], f32)) for i in range(2)]

    xT_view = big[:, :SHARD]
    s_view = big[0:2, SHARD:2 * SHARD]
    tab_view = big[:, :CH]

    n_mm = (SHARD + MMT - 1) // MMT  # 25 (24x512 + 256)

    with (
        nc.semaphore("is_") as is_,     # input DMAs
        nc.semaphore("pe") as pe,       # matmul tiles
        nc.semaphore("ve") as ve,       # psum evacuations
        nc.semaphore("sl") as sl,       # s -> dram
        nc.semaphore("cc") as cc,       # collective
        nc.semaphore("ts") as ts,       # table DMAs
        nc.semaphore("gs") as gs,       # gathers
        nc.semaphore("rs") as rs,       # reshuffle DMAs
        nc.semaphore("ms") as ms,       # mask DMAs
        nc.semaphore("vs") as vs,       # DVE combine per tile
        nc.semaphore("asm") as asm,     # ACT sigmoid per tile
        nc.semaphore("os_") as os_,     # output DMAs
        nc.Block() as block,
    ):
        @block.sync
        def _(sync):
            sync.dma_start(out=xT_view, in_=xT_ext[:]).then_inc(is_, 16)
            sync.dma_start(out=w_sb[:], in_=w_ext[:]).then_inc(is_, 16)
            sync.dma_start(out=idxr_sb[:], in_=idxr_ext[:]).then_inc(is_, 16)
            sync.dma_start(out=idxc_sb[:], in_=idxc_ext[:]).then_inc(is_, 16)

            for c in range(1, 4):
                sync.dma_start(out=mR[0][32 * c:32 * c + 8, :], in_=mask_ext[c - 1, :, 0:TG]).then_inc(ms, 16)
                sync.dma_start(out=mC[0][32 * c:32 * c + 8, :], in_=mask_ext[c + 2, :, 0:TG]).then_inc(ms, 16)

            # s table to DRAM once computed
            sync.wait_ge(ve, n_mm)
            sync.dma_start(out=s_loc[:], in_=s_view).then_inc(sl, 16)

            # after the collective, broadcast sub-tables into SBUF:
            # partition p holds sub-table c=p%8; c<4: row chunk c (s_all rows
            # 4c, 4c+2); c>=4: col chunk c-4 (s_all rows 4(c-4)+1, 4(c-4)+3).
            sync.wait_ge(cc, 1)
            from concourse.ap import AP as _AP
            for c in range(8):
                r0 = 4 * c if c < 4 else 4 * (c - 4) + 1
                src = _AP(
                    s_all[:].tensor, r0 * SHARD,
                    [[0, 16], [2 * SHARD, 2], [1, SHARD]],
                )
                dst = big[c::8, :CH].rearrange("p (a b) -> p a b", a=2)
                sync.dma_start(out=dst, in_=src).then_inc(ts, 16)

            # steady-state tile loop
            for t in range(TIL):
                b = t % 2
                # output DMA for tile t-1
                if t > 0:
                    sync.wait_ge(asm, t)
                    sync.dma_start(
                        out=out_ext[:, TG * (t - 1):TG * t], in_=sig[(t - 1) % 2][:]
                    ).then_inc(os_, 16)
                # prefetch masks for tile t+1 (safe: asm>=t ensures DVE(t-1) done)
                if t + 1 < TIL:
                    for c in range(1, 4):
                        sync.dma_start(out=mR[(t + 1) % 2][32 * c:32 * c + 8, :], in_=mask_ext[c - 1, :, TG * (t + 1):TG * (t + 2)]).then_inc(ms, 16)
                        sync.dma_start(out=mC[(t + 1) % 2][32 * c:32 * c + 8, :], in_=mask_ext[c + 2, :, TG * (t + 1):TG * (t + 2)]).then_inc(ms, 16)
                # reshuffle candidates of tile t into 32-aligned blocks
                sync.wait_ge(gs, 2 * (t + 1))
                if t > 0:
                    sync.wait_ge(asm, t)  # blk buffers free after ACT(t-1)
                for c in range(4):
                    sync.dma_start(out=blkR[32 * c:32 * c + 8, :], in_=outR[b][c::16, :]).then_inc(rs, 16)
                    sync.dma_start(out=blkC[32 * c:32 * c + 8, :], in_=outC[b][(4 + c)::16, :]).then_inc(rs, 16)
            # final output DMA
            sync.wait_ge(asm, TIL)
            sync.dma_start(
                out=out_ext[:, TG * (TIL - 1):TG * TIL], in_=sig[(TIL - 1) % 2][:]
            ).then_inc(os_, 16)
            sync.wait_ge(os_, 16 * TIL)

        @block.tensor
        def _(tensor):
            tensor.wait_ge(is_, 64)
            for k in range(n_mm):
                lo = MMT * k
                hi = min(SHARD, lo + MMT)
                if k >= 2:
                    tensor.wait_ge(ve, k - 1)
                tensor.matmul(
                    psum[k % 2][:, :hi - lo], w_sb[:], xT_view[:, lo:hi],
                    start=True, stop=True,
                ).then_inc(pe, 1)

        @block.vector
        def _(vector):
            for k in range(n_mm):
                lo = MMT * k
                hi = min(SHARD, lo + MMT)
                vector.wait_ge(pe, k + 1)
                vector.tensor_copy(s_view[:, lo:hi], psum[k % 2][:, :hi - lo]).then_inc(ve, 1)
            # combine loop
            for t in range(TIL):
                vector.wait_ge(rs, 128 * (t + 1))
                vector.wait_ge(ms, 96 * (t + 1))
                for c in range(1, 4):
                    vector.copy_predicated(blkR[0:8, :], mR[t % 2][32 * c:32 * c + 8, :], blkR[32 * c:32 * c + 8, :])
                    vector.copy_predicated(blkC[0:8, :], mC[t % 2][32 * c:32 * c + 8, :], blkC[32 * c:32 * c + 8, :])
                vector.tensor_tensor(
                    out=blkR[0:8, :], in0=blkR[0:8, :], in1=blkC[0:8, :],
                    op=mybir.AluOpType.add,
                ).then_inc(vs, 1)

        @block.scalar
        def _(scalar):
            for t in range(TIL):
                scalar.wait_ge(vs, t + 1)
                if t >= 2:
                    scalar.wait_ge(os_, 16 * (t - 1))
                scalar.activation(
                    sig[t % 2][:], blkR[0:8, :], mybir.ActivationFunctionType.Sigmoid
                ).then_inc(asm, 1)

        @block.gpsimd
        def _(gpsimd):
            gpsimd.wait_ge(sl, 16)
            gpsimd.collective_compute(
                "AllGather",
                mybir.AluOpType.bypass,
                replica_groups=[list(range(N_CORES))],
                ins=[s_loc[:]],
                outs=[s_all[:]],
            ).then_inc(cc, 1)
            gpsimd.wait_ge(ts, 16 * 8)
            gpsimd.wait_ge(is_, 64)
            for t in range(TIL):
                b = t % 2
                if t >= 2:
                    gpsimd.wait_ge(rs, 128 * (t - 1))
                gpsimd.ap_gather(
                    outR[b][:], tab_view, idxr_sb[:, TGC * t:TGC * (t + 1)],
                    channels=128, num_elems=CH, d=1, num_idxs=TG,
                ).then_inc(gs, 1)
                gpsimd.ap_gather(
                    outC[b][:], tab_view, idxc_sb[:, TGC * t:TGC * (t + 1)],
                    channels=128, num_elems=CH, d=1, num_idxs=TG,
                ).then_inc(gs, 1)

    nc.finalize()
    return nc


def _wrap(arr):
    # [8, GEDGE] per-group streams -> ap_gather wrapped layout [128, GEDGE//16]
    return (
        arr.reshape(8, GEDGE // 16, 16).transpose(0, 2, 1).reshape(128, GEDGE // 16)
    )


def kernel(edge_index, x, att_weight):
    global LAST_EXEC_NS
    from concourse.bass_utils import run_bass_kernel_spmd

    if "nc" not in _CACHE:
        _CACHE["nc"] = _build()
    nc = _CACHE["nc"]

    x = np.asarray(x, dtype=np.float32)
    att_weight = np.asarray(att_weight, dtype=np.float32)
    ei = np.asarray(edge_index)

    xT = np.zeros((H, NTOT), dtype=np.float32)
    xT[:, :N_NODES] = x.T
    w = np.empty((H, 2), dtype=np.float32)
    w[:, 0] = att_weight[0, :H]
    w[:, 1] = att_weight[0, H:]

    in_maps = []
    for c in range(N_CORES):
        sl = slice(EPC * c, EPC * (c + 1))
        r = np.zeros(PADE, dtype=np.int64)
        co = np.zeros(PADE, dtype=np.int64)
        r[:EPC] = ei[0, sl]
        co[:EPC] = ei[1, sl]
        rch = (r // CH).astype(np.uint8).reshape(8, GEDGE)
        cch = (co // CH).astype(np.uint8).reshape(8, GEDGE)
        rloc = (r % CH).astype(np.int16).reshape(8, GEDGE)
        cloc = (co % CH).astype(np.int16).reshape(8, GEDGE)
        mask = np.zeros((6, 8, GEDGE), dtype=np.uint8)
        for cc_ in range(1, 4):
            mask[cc_ - 1] = rch == cc_
            mask[cc_ + 2] = cch == cc_
        in_maps.append({
            "xT": np.ascontiguousarray(xT[:, SHARD * c:SHARD * (c + 1)]),
            "w": w,
            "idxr": np.ascontiguousarray(_wrap(rloc)),
            "idxc": np.ascontiguousarray(_wrap(cloc)),
            "mask": mask,
        })

    trace = bool(os.environ.get("BASS_TRACE"))
    try:
        res = run_bass_kernel_spmd(nc, in_maps, list(range(N_CORES)), trace=trace)
    except Exception:
        if not trace:
            raise
        res = run_bass_kernel_spmd(nc, in_maps, list(range(N_CORES)), trace=False)
    LAST_EXEC_NS = res.exec_time_ns

    parts = [res.results[c]["out"].reshape(-1)[:EPC] for c in range(N_CORES)]
    return np.concatenate(parts).astype(np.float32)[:, None]
